# revision 49
# baseline (speedup 1.0000x reference)
"""Causal attention (QKV proj + softmax + PV + ReLU) on 8 trn2 NeuronCores.

Sharding: data-parallel over batch B=32 -> 4 batches per core; projection
weights replicated.

Numerics: fp8-e4m3 with perf_mode=DoubleRow (contraction 256 per pass,
~1.8x the fp16 PE issue rate; N=512 DR matmuls measured at ~215-231 ns,
LDWEIGHTS fully hidden by the PE reorder window) for the bulk of the FLOPs;
fp32 PSUM accumulation. Max-norm accuracy is protected by small fp16
islands for the early rows, where softmax averages over few values and fp8
noise would not cancel (numpy-simulated: full-fp8 rel err 6.1e-2 FAILs;
fp8 with islands 7.5e-3; measured on HW 7.57e-3 vs the 2e-2 gate):
  - rows i<128 get fp16 S from fp16-PROJECTED q,k islands (the dominant
    term is x8/W8 projection noise, so re-draining the fp8 psum at fp16
    is NOT enough), fp16 P00, and a fp16 V tile
  - everything else runs q8/k8/p8/v8 through DoubleRow pairs
The softmax rowsum is computed from the SAME quantized P used for P@V, so
uniform quantization error cancels in the normalize.

Per core, per batch:
  Q^T,K^T[d,l] = W8^T.T @ X8^T   (4 DoubleRow matmuls, 256-contraction each;
      bias folded into the ACT Identity drain: per-partition bias = b[d])
  V[l,d] = X8^T.T @ Wv8^T        (DoubleRow; bias via DVE add during drain)
  S^T[j,i] tiles = K8^T.T @ Q8^T  (2 DoubleRow matmuls per tile; causal:
      chunks start at the diagonal; (j<128,i<128) tile comes from fp16)
  P^T = exp(scale*S^T + padmask_bias_j) via one ACT op -> fp8 pair tiles;
      diagonal tile masked by a DVE multiply; the causal-zero 128-col strip
      of odd planes is memset so even output rows can consume full pairs
  O' = P8^T.T @ V8 pairs, rowsum = P8^T.T @ ones8 piggybacks on the same
      stationary operand (LDWEIGHTS deduped)
  out = Relu(O' * (1/rowsum)) on ACT (per-partition scale), fp16, DMA out

Engine balance (NTFF-verified): Q/K proj drains + exps + half the PV
normalizes on ACT (per-partition bias/scale), V drains + diag masks +
reciprocals + the other normalizes on DVE, stores + P-zero memsets on
gpsimd, x prefetch on sync, weights on scalar -- no engine above ~65% of
the PE's fp8 critical path. PSUM: proj pool 3 banks, S 2, PV-O 2,
rowsum 1 (rowsum drains fast via reciprocal, so 1 bank suffices; the
freed bank decouples proj drains from the matmul stream). o_sb bufs=6 so
the PV normalize burst never waits on store DMAs. Warmup matmuls pre-warm
the HAM clock-gate while wq8 (split per-group over the scalar+gpsimd
rings) and batch-0 x lc0-halves (sync+gpsimd) stream in.
"""

import os
from contextlib import ExitStack

import numpy as np
import ml_dtypes

import concourse.tile as tile
from concourse import bacc, mybir
from concourse import bass_utils

F32 = mybir.dt.float32
F16 = mybir.dt.float16
F8 = mybir.dt.float8e4
AF = mybir.ActivationFunctionType
DR = mybir.MatmulPerfMode.DoubleRow
E4NP = ml_dtypes.float8_e4m3

N_CORES = 8
B = 32
L = 1024
C = 1024  # d_model
D = 512
P = 128
NB = B // N_CORES  # batches per core
CT = C // P  # 8 c-tiles
G = C // 256  # 4 DoubleRow contraction groups over c
DT = D // P  # 4 d tiles
G2 = D // 256  # 2 DoubleRow groups over d
LT = L // P  # 8 l/j/i tiles
GL = LT // 2  # 4 j-pair groups
SCALE = float(D) ** -0.5
NEG = -30000.0


def build_program(nb: int = NB):
    """Build the per-core Bass program for nb batches."""
    nc = bacc.Bacc("TRN2", target_bir_lowering=False, debug=False,
                   num_devices=N_CORES)

    # [b, g, p, i, l] fp8 pairs of x^T; plane i pairs c = 256g+128i+p
    x8d = nc.dram_tensor("x8d", [nb, G, P, 2, L], F8, kind="ExternalInput").ap()
    # [b, ct, p, l<128] fp16 x^T for the l<128 fp16 V island
    x16d = nc.dram_tensor("x16d", [nb, CT, P, P], F16, kind="ExternalInput").ap()
    wq8 = nc.dram_tensor("wq8", [G, P, 2, D], F8, kind="ExternalInput").ap()
    wk8 = nc.dram_tensor("wk8", [G, P, 2, D], F8, kind="ExternalInput").ap()
    wv8 = nc.dram_tensor("wv8", [G, P, 2, D], F8, kind="ExternalInput").ap()
    wv16 = nc.dram_tensor("wv16", [CT, P, D], F16, kind="ExternalInput").ap()
    wq16 = nc.dram_tensor("wq16", [CT, P, D], F16, kind="ExternalInput").ap()
    wk16 = nc.dram_tensor("wk16", [CT, P, D], F16, kind="ExternalInput").ap()
    bq2 = nc.dram_tensor("bq2", [P, DT], F32, kind="ExternalInput").ap()
    bk2 = nc.dram_tensor("bk2", [P, DT], F32, kind="ExternalInput").ap()
    bvb = nc.dram_tensor("bvb", [P, D], F32, kind="ExternalInput").ap()
    pmt = nc.dram_tensor("pmt", [nb, P, LT], F32, kind="ExternalInput").ap()
    tri8d = nc.dram_tensor("tri8d", [P, P], F8, kind="ExternalInput").ap()
    tri16d = nc.dram_tensor("tri16d", [P, P], F16, kind="ExternalInput").ap()
    out = nc.dram_tensor("out", [nb, L, D], F16, kind="ExternalOutput").ap()

    with tile.TileContext(nc) as tc, ExitStack() as ctx:
        const = ctx.enter_context(tc.tile_pool(name="const", bufs=1))
        xt_pool = ctx.enter_context(tc.tile_pool(name="xt", bufs=3))
        qk_pool = ctx.enter_context(tc.tile_pool(name="qk", bufs=2))
        qk16_pool = ctx.enter_context(tc.tile_pool(name="qk16", bufs=2))
        v_pool = ctx.enter_context(tc.tile_pool(name="v", bufs=2))
        pt_pool = ctx.enter_context(tc.tile_pool(name="pt", bufs=2))
        o_pool = ctx.enter_context(tc.tile_pool(name="o", bufs=6))
        sm_pool = ctx.enter_context(tc.tile_pool(name="sm", bufs=4))
        pm_pool = ctx.enter_context(tc.tile_pool(name="pm", bufs=2))
        proj_ps = ctx.enter_context(tc.tile_pool(name="pps", bufs=3, space="PSUM"))
        s_ps = ctx.enter_context(tc.tile_pool(name="sps", bufs=2, space="PSUM"))
        o_ps = ctx.enter_context(tc.tile_pool(name="ops", bufs=2, space="PSUM"))
        r_ps = ctx.enter_context(tc.tile_pool(name="rps", bufs=1, space="PSUM"))

        # --- constants; on the scalar HWDGE queue so the sync queue is
        # dedicated to x prefetch ---
        wq_sb = const.tile([P, G, 2, D], F8)
        # startup-critical: wq and batch-0 x(lc0) split per-group across all
        # three DMA rings so the first Q group streams in ~4us earlier
        for g in range(3):
            nc.scalar.dma_start(wq_sb[:, g:g + 1],
                                wq8[g:g + 1].rearrange("g p i d -> p g i d"))
        nc.gpsimd.dma_start(wq_sb[:, 3:4],
                            wq8[3:4].rearrange("g p i d -> p g i d"))
        wk_sb = const.tile([P, G, 2, D], F8)
        wv_sb = const.tile([P, G, 2, D], F8)
        wv16_sb = const.tile([P, CT, D], F16)
        wq16_sb = const.tile([P, CT, D], F16)
        wk16_sb = const.tile([P, CT, D], F16)
        bq_sb = const.tile([P, DT], F32)
        nc.scalar.dma_start(bq_sb[:], bq2[:])
        bk_sb = const.tile([P, DT], F32)
        nc.scalar.dma_start(bk_sb[:], bk2[:])
        bv_sb = const.tile([P, D], F32)
        nc.scalar.dma_start(bv_sb[:], bvb[:])
        tri8_sb = const.tile([P, P], F8)
        nc.scalar.dma_start(tri8_sb[:], tri8d[:])
        tri16_sb = const.tile([P, P], F16)
        nc.scalar.dma_start(tri16_sb[:], tri16d[:])

        ones8_sb = const.tile([P, 2, 1], F8)
        nc.vector.memset(ones8_sb[:], 1.0)
        ones16_sb = const.tile([P, 1], F16)
        nc.vector.memset(ones16_sb[:], 1.0)

        # PE warmup: dummy matmuls with no input deps keep the PE busy while
        # batch-0 inputs stream in (HAM clock-gate to 2.4 GHz).
        warm_sb = const.tile([P, 512], F16)
        nc.vector.memset(warm_sb[:], 0.0)
        for w in range(8):
            wps = proj_ps.tile([P, 512], F32, tag="pp", name=f"warm{w}")
            nc.tensor.matmul(wps[:], warm_sb[:, 0:P], warm_sb[:],
                             start=True, stop=True)

        for b in range(nb):
            # --- X^T fp8 pair tiles [128, 2, 1024] ---
            xt = []
            if b == 0:
                # first batch: load the l<512 halves of every tile first so
                # the Q lc=0 groups can start earlier; spread across the
                # sync + gpsimd rings to halve the critical prefetch time
                for g in range(G):
                    t = xt_pool.tile([P, 2, L], F8, tag=f"xt{g}",
                                     name=f"xt{g}_{b}")
                    eng = nc.sync if g < 3 else nc.gpsimd
                    eng.dma_start(t[:, :, 0:512], x8d[b, g, :, :, 0:512])
                    xt.append(t)
                for g in range(G):
                    nc.sync.dma_start(xt[g][:, :, 512:L],
                                      x8d[b, g, :, :, 512:L])
            else:
                for g in range(G):
                    t = xt_pool.tile([P, 2, L], F8, tag=f"xt{g}",
                                     name=f"xt{g}_{b}")
                    nc.sync.dma_start(t[:], x8d[b, g])
                    xt.append(t)
            # fp16 x^T tiles for the l<128 fp16 V island: [128, ct, 128]
            xt16 = xt_pool.tile([P, CT, P], F16, tag="xt16", name=f"xt16_{b}")
            nc.sync.dma_start(xt16[:], x16d[b].rearrange("t p l -> p t l"))
            pm_sb = pm_pool.tile([P, LT], F32, name=f"pm_{b}")
            nc.sync.dma_start(pm_sb[:], pmt[b])
            if b == 0:
                # behind batch-0 x on the sync ring: don't steal HBM bw from
                # the startup-critical x prefetch
                nc.sync.dma_start(wk_sb[:], wk8.rearrange("g p i d -> p g i d"))
                nc.sync.dma_start(wv_sb[:], wv8.rearrange("g p i d -> p g i d"))
                nc.sync.dma_start(wv16_sb[:],
                                  wv16.rearrange("t p d -> p t d"))
                nc.sync.dma_start(wq16_sb[:],
                                  wq16.rearrange("t p d -> p t d"))
                nc.sync.dma_start(wk16_sb[:],
                                  wk16.rearrange("t p d -> p t d"))

            # --- Q^T, K^T: fp8 pair tiles [128, 2, 1024] per d-group;
            # plus fp16 [128d, 128l] tiles of the l<128 slice for S00 ---
            q8t, k8t = [], []
            q16t, k16t = [], []
            for name, w_sb, b_sb, d8 in (("q", wq_sb, bq_sb, q8t),
                                         ("k", wk_sb, bk_sb, k8t)):
                for g2 in range(G2):
                    d8.append(qk_pool.tile([P, 2, L], F8, tag=f"{name}8{g2}",
                                           name=f"{name}8{g2}_{b}"))
                if b == 0 and name == "q":
                    order = [(dt, lc) for lc in range(2) for dt in range(DT)]
                else:
                    order = [(dt, lc) for dt in range(DT) for lc in range(2)]
                for dt, lc in order:
                    ps = proj_ps.tile([P, 512], F32, tag="pp",
                                      name=f"{name}ps{dt}_{lc}_{b}")
                    for g in range(G):
                        nc.tensor.matmul(
                            ps[:],
                            w_sb[:, g, :, dt * P:(dt + 1) * P],
                            xt[g][:, :, lc * 512:(lc + 1) * 512],
                            start=(g == 0), stop=(g == G - 1),
                            perf_mode=DR)
                    # drain on ACT: Identity(ps + bias[d]) -> fp8 pair slice
                    nc.scalar.activation(
                        d8[dt // 2][:, dt % 2, lc * 512:(lc + 1) * 512],
                        ps[:], AF.Identity, bias=b_sb[:, dt:dt + 1])
            # fp16 q,k islands (l<128) from the fp16 path: clean logits for
            # the small-n softmax rows (fp8 proj noise would not cancel)
            for name, w16_sb, b_sb, d16 in (("q", wq16_sb, bq_sb, q16t),
                                            ("k", wk16_sb, bk_sb, k16t)):
                for dt in range(DT):
                    t = qk16_pool.tile([P, P], F16, tag=f"{name}16{dt}",
                                       name=f"{name}16{dt}_{b}")
                    d16.append(t)
                    ps = proj_ps.tile([P, P], F32, tag="pp",
                                      name=f"{name}16ps{dt}_{b}")
                    for ct in range(CT):
                        nc.tensor.matmul(ps[:],
                                         w16_sb[:, ct, dt * P:(dt + 1) * P],
                                         xt16[:, ct, :],
                                         start=(ct == 0), stop=(ct == CT - 1))
                    nc.scalar.activation(t[:], ps[:], AF.Identity,
                                         bias=b_sb[:, dt:dt + 1])

            # --- V: fp8 pair tiles [128, 2, 512] per j-pair; bias on DVE ---
            v8 = []
            for g in range(GL):
                v8.append(v_pool.tile([P, 2, D], F8, tag=f"v{g}",
                                      name=f"v{g}_{b}"))
            for lt in range(LT):
                ps = proj_ps.tile([P, D], F32, tag="pp", name=f"vps{lt}_{b}")
                for g in range(G):
                    nc.tensor.matmul(ps[:], xt[g][:, :, lt * P:(lt + 1) * P],
                                     wv_sb[:, g, :, :],
                                     start=(g == 0), stop=(g == G - 1),
                                     perf_mode=DR)
                nc.vector.tensor_add(v8[lt // 2][:, lt % 2, :], ps[:], bv_sb[:])
            # fp16 V island (rows j<128) from the fp16 path
            v16 = v_pool.tile([P, D], F16, tag="v16", name=f"v16_{b}")
            ps = proj_ps.tile([P, D], F32, tag="pp", name=f"v16ps_{b}")
            for ct in range(CT):
                nc.tensor.matmul(ps[:], xt16[:, ct, :], wv16_sb[:, ct, :],
                                 start=(ct == 0), stop=(ct == CT - 1))
            nc.vector.tensor_add(v16[:], ps[:], bv_sb[:])

            # --- S^T tiles + exp -> P^T fp8 pairs (causal) ---
            p8 = []
            for g in range(GL):
                p8.append(pt_pool.tile([P, 2, L], F8, tag=f"p8{g}",
                                       name=f"p8{g}_{b}"))
            # fp16 S00 island: S^T[j<128, i<128]
            pt16 = pt_pool.tile([P, P], F16, tag="pt16", name=f"pt16_{b}")
            ps = s_ps.tile([P, P], F32, tag="sp", name=f"s00_{b}")
            for dt in range(DT):
                nc.tensor.matmul(ps[:], k16t[dt][:], q16t[dt][:],
                                 start=(dt == 0), stop=(dt == DT - 1))
            nc.scalar.activation(pt16[:], ps[:], AF.Exp,
                                 bias=pm_sb[:, 0:1], scale=SCALE)
            nc.vector.tensor_mul(pt16[:], pt16[:], tri16_sb[:])

            for jb in range(LT):
                j0 = jb * P
                i0 = j0 if jb > 0 else P
                while i0 < L:
                    n = min((i0 // 512 + 1) * 512, L) - i0
                    ps = s_ps.tile([P, n], F32, tag="sp",
                                   name=f"sps{jb}_{i0}_{b}")
                    for g2 in range(G2):
                        nc.tensor.matmul(ps[:],
                                         k8t[g2][:, :, j0:j0 + P],
                                         q8t[g2][:, :, i0:i0 + n],
                                         start=(g2 == 0), stop=(g2 == G2 - 1),
                                         perf_mode=DR)
                    nc.scalar.activation(p8[jb // 2][:, jb % 2, i0:i0 + n],
                                         ps[:], AF.Exp,
                                         bias=pm_sb[:, jb:jb + 1], scale=SCALE)
                    i0 += n
                if jb > 0:
                    # mask the diagonal tile: keep j<=i
                    nc.vector.tensor_mul(p8[jb // 2][:, jb % 2, j0:j0 + P],
                                         p8[jb // 2][:, jb % 2, j0:j0 + P],
                                         tri8_sb[:])
            # zero the causal strips odd planes contribute to even rows:
            # plane jb=2g+1, i-block 2g is never written but IS consumed
            # when output block ib=2g>=2 runs the full pair
            for g in range(1, GL):
                nc.gpsimd.memset(p8[g][:, 1, 2 * g * P:(2 * g + 1) * P], 0.0)

            # --- O' = P^T.T @ V pairs + rowsum piggyback ---
            # all 8 ib rowsums share ONE psum bank, one column each: the
            # accumulation groups hit disjoint [P,1] regions, so ib's
            # rowsum-start never WARs on ib-1's reciprocal (was ~450ns
            # stall, 4x per batch)
            rps = r_ps.tile([P, LT], F32, tag="rp", name=f"rps_{b}")
            for ib in range(LT):
                i0 = ib * P
                rcol = rps[:, ib:ib + 1]
                ops = o_ps.tile([P, D], F32, tag="op", name=f"ops{ib}_{b}")
                if ib == 0:
                    nc.tensor.matmul(ops[:], pt16[:], v16[:],
                                     start=True, stop=True)
                    nc.tensor.matmul(rcol, pt16[:], ones16_sb[:],
                                     start=True, stop=True)
                else:
                    ng = (ib + 2) // 2  # pairs covering j-tiles 0..ib (+pad)
                    for g in range(ng):
                        pT = p8[g][:, :, i0:i0 + P]
                        nc.tensor.matmul(ops[:], pT, v8[g][:],
                                         start=(g == 0), stop=(g == ng - 1),
                                         perf_mode=DR)
                        nc.tensor.matmul(rcol, pT, ones8_sb[:],
                                         start=(g == 0), stop=(g == ng - 1),
                                         perf_mode=DR)
                rec = sm_pool.tile([P, 1], F32, tag="rec", name=f"rec{ib}_{b}")
                nc.vector.reciprocal(rec[:], rcol)
                o_sb = o_pool.tile([P, D], F16, tag="ot", name=f"o{ib}_{b}")
                # relu(O'/rowsum): alternate ACT/DVE so the 8-deep normalize
                # burst at the PV tail doesn't queue on one engine and gate
                # the o-psum rotation
                if ib % 2 == 0:
                    nc.scalar.activation(o_sb[:], ops[:], AF.Relu,
                                         scale=rec[:, 0:1])
                else:
                    nc.vector.tensor_scalar(o_sb[:], ops[:], rec[:], 0.0,
                                            mybir.AluOpType.mult,
                                            mybir.AluOpType.max)
                # SWDGE so stores never head-of-line-block the x prefetch;
                # last batch has no prefetch left, so use the two idle HWDGE
                # rings alternately to halve the final store drain
                if b == nb - 1:
                    eng = nc.sync if ib % 2 == 0 else nc.scalar
                    eng.dma_start(out[b, i0:i0 + P, :], o_sb[:])
                else:
                    nc.gpsimd.dma_start(out[b, i0:i0 + P, :], o_sb[:])

    nc.compile()
    return nc


def _prep_host(x, Wq, bq, Wk, bk, Wv, bv, mask):
    # x^T pairs: x8d[b, g, p, i, l] = x[b, l, 256g+128i+p] in e4m3
    x8 = x.astype(E4NP)
    x8d = np.ascontiguousarray(
        x8.transpose(0, 2, 1).reshape(B, G, 2, P, L).transpose(0, 1, 3, 2, 4))
    x16d = np.ascontiguousarray(
        x[:, 0:P, :].astype(np.float16).transpose(0, 2, 1).reshape(
            B, CT, P, P))
    # weights: w8[g, p, i, d] = W[d, 256g+128i+p]
    def w8of(W):
        return np.ascontiguousarray(
            W.T.astype(E4NP).reshape(G, 2, P, D).transpose(0, 2, 1, 3))
    wq8 = w8of(Wq)
    wk8 = w8of(Wk)
    wv8 = w8of(Wv)
    wv16 = np.ascontiguousarray(
        Wv.T.astype(np.float16).reshape(CT, P, D))
    wq16 = np.ascontiguousarray(
        Wq.T.astype(np.float16).reshape(CT, P, D))
    wk16 = np.ascontiguousarray(
        Wk.T.astype(np.float16).reshape(CT, P, D))
    bq2 = np.ascontiguousarray(bq.astype(np.float32).reshape(DT, P).T)
    bk2 = np.ascontiguousarray(bk.astype(np.float32).reshape(DT, P).T)
    bvb = np.ascontiguousarray(
        np.broadcast_to(bv.astype(np.float32), (P, D)))
    pm = np.where(mask[:, 0, :] != 0, 0.0, NEG).astype(np.float32)  # [B, L]
    pmt = np.ascontiguousarray(pm.reshape(B, LT, P).transpose(0, 2, 1))
    tri = (np.arange(P)[:, None] <= np.arange(P)[None, :])
    tri8 = tri.astype(E4NP)
    tri16 = tri.astype(np.float16)
    return (x8d, x16d, wq8, wk8, wv8, wv16, wq16, wk16, bq2, bk2, bvb, pmt,
            tri8, tri16)


_NC_CACHE = {}


def kernel(x, Wq, bq, Wk, bk, Wv, bv, mask):
    x = np.asarray(x)
    Wq, bq = np.asarray(Wq), np.asarray(bq)
    Wk, bk = np.asarray(Wk), np.asarray(bk)
    Wv, bv = np.asarray(Wv), np.asarray(bv)
    mask = np.asarray(mask)

    (x8d, x16d, wq8, wk8, wv8, wv16, wq16, wk16, bq2, bk2, bvb, pmt, tri8,
     tri16) = _prep_host(x, Wq, bq, Wk, bk, Wv, bv, mask)

    if "nc" not in _NC_CACHE:
        _NC_CACHE["nc"] = build_program(NB)
    nc = _NC_CACHE["nc"]

    in_maps = []
    for c in range(N_CORES):
        s = slice(c * NB, (c + 1) * NB)
        in_maps.append({
            "x8d": np.ascontiguousarray(x8d[s]),
            "x16d": np.ascontiguousarray(x16d[s]),
            "wq8": wq8, "wk8": wk8, "wv8": wv8, "wv16": wv16,
            "wq16": wq16, "wk16": wk16,
            "bq2": bq2, "bk2": bk2, "bvb": bvb,
            "pmt": np.ascontiguousarray(pmt[s]),
            "tri8d": tri8, "tri16d": tri16,
        })

    res = bass_utils.run_bass_kernel_spmd(
        nc, in_maps, core_ids=list(range(N_CORES)),
        trace=bool(int(os.environ.get("KERNEL_TRACE", "0"))),
    )
    if os.environ.get("KERNEL_RESULT_HOOK"):
        _NC_CACHE["last_result"] = res

    return np.concatenate(
        [res.results[c]["out"] for c in range(N_CORES)],
        axis=0).astype(np.float32)


# revision 50
# speedup vs baseline: 1.0118x; 1.0118x over previous
"""Causal attention (QKV proj + softmax + PV + ReLU) on 8 trn2 NeuronCores.

Sharding: data-parallel over batch B=32 -> 4 batches per core; projection
weights replicated.

Numerics: fp8-e4m3 with perf_mode=DoubleRow (contraction 256 per pass,
~1.8x the fp16 PE issue rate; N=512 DR matmuls measured at ~215-231 ns,
LDWEIGHTS fully hidden by the PE reorder window) for the bulk of the FLOPs;
fp32 PSUM accumulation. Max-norm accuracy is protected by small fp16
islands for the early rows, where softmax averages over few values and fp8
noise would not cancel (numpy-simulated: full-fp8 rel err 6.1e-2 FAILs;
fp8 with islands 7.5e-3; measured on HW 7.57e-3 vs the 2e-2 gate):
  - rows i<128 get fp16 S from fp16-PROJECTED q,k islands (the dominant
    term is x8/W8 projection noise, so re-draining the fp8 psum at fp16
    is NOT enough), fp16 P00, and a fp16 V tile
  - everything else runs q8/k8/p8/v8 through DoubleRow pairs
The softmax rowsum is computed from the SAME quantized P used for P@V, so
uniform quantization error cancels in the normalize.

Per core, per batch:
  Q^T,K^T[d,l] = W8^T.T @ X8^T   (4 DoubleRow matmuls, 256-contraction each;
      bias folded into the ACT Identity drain: per-partition bias = b[d])
  V[l,d] = X8^T.T @ Wv8^T        (DoubleRow; bias via DVE add during drain)
  S^T[j,i] tiles = K8^T.T @ Q8^T  (2 DoubleRow matmuls per tile; causal:
      chunks start at the diagonal; (j<128,i<128) tile comes from fp16)
  P^T = exp(scale*S^T + padmask_bias_j) via one ACT op -> fp8 pair tiles;
      diagonal tile masked by a DVE multiply; the causal-zero 128-col strip
      of odd planes is memset so even output rows can consume full pairs
  O' = P8^T.T @ V8 pairs, rowsum = P8^T.T @ ones8 piggybacks on the same
      stationary operand (LDWEIGHTS deduped)
  out = Relu(O' * (1/rowsum)) on ACT (per-partition scale), fp16, DMA out

Engine balance (NTFF-verified): Q/K proj drains + exps + half the PV
normalizes on ACT (per-partition bias/scale), V drains + diag masks +
reciprocals + the other normalizes on DVE, stores + P-zero memsets on
gpsimd, x prefetch on sync, weights on scalar -- no engine above ~65% of
the PE's fp8 critical path. PSUM: proj pool 3 banks, S 2, PV-O 2,
rowsum 1 (rowsum drains fast via reciprocal, so 1 bank suffices; the
freed bank decouples proj drains from the matmul stream). o_sb bufs=6 so
the PV normalize burst never waits on store DMAs. Warmup matmuls pre-warm
the HAM clock-gate while wq8 (split per-group over the scalar+gpsimd
rings) and batch-0 x lc0-halves (sync+gpsimd) stream in.
"""

import os
from contextlib import ExitStack

import numpy as np
import ml_dtypes

import concourse.tile as tile
from concourse import bacc, mybir
from concourse import bass_utils

F32 = mybir.dt.float32
F16 = mybir.dt.float16
F8 = mybir.dt.float8e4
AF = mybir.ActivationFunctionType
DR = mybir.MatmulPerfMode.DoubleRow
E4NP = ml_dtypes.float8_e4m3

N_CORES = 8
B = 32
L = 1024
C = 1024  # d_model
D = 512
P = 128
NB = B // N_CORES  # batches per core
CT = C // P  # 8 c-tiles
G = C // 256  # 4 DoubleRow contraction groups over c
DT = D // P  # 4 d tiles
G2 = D // 256  # 2 DoubleRow groups over d
LT = L // P  # 8 l/j/i tiles
GL = LT // 2  # 4 j-pair groups
SCALE = float(D) ** -0.5
NEG = -30000.0


def build_program(nb: int = NB):
    """Build the per-core Bass program for nb batches."""
    nc = bacc.Bacc("TRN2", target_bir_lowering=False, debug=False,
                   num_devices=N_CORES)

    # [b, g, p, i, l] fp8 pairs of x^T; plane i pairs c = 256g+128i+p
    x8d = nc.dram_tensor("x8d", [nb, G, P, 2, L], F8, kind="ExternalInput").ap()
    # [b, ct, p, l<128] fp16 x^T for the l<128 fp16 V island
    x16d = nc.dram_tensor("x16d", [nb, CT, P, P], F16, kind="ExternalInput").ap()
    wq8 = nc.dram_tensor("wq8", [G, P, 2, D], F8, kind="ExternalInput").ap()
    wk8 = nc.dram_tensor("wk8", [G, P, 2, D], F8, kind="ExternalInput").ap()
    wv8 = nc.dram_tensor("wv8", [G, P, 2, D], F8, kind="ExternalInput").ap()
    wv16 = nc.dram_tensor("wv16", [CT, P, D], F16, kind="ExternalInput").ap()
    wq16 = nc.dram_tensor("wq16", [CT, P, D], F16, kind="ExternalInput").ap()
    wk16 = nc.dram_tensor("wk16", [CT, P, D], F16, kind="ExternalInput").ap()
    bq2 = nc.dram_tensor("bq2", [P, DT], F32, kind="ExternalInput").ap()
    bk2 = nc.dram_tensor("bk2", [P, DT], F32, kind="ExternalInput").ap()
    bvb = nc.dram_tensor("bvb", [P, D], F32, kind="ExternalInput").ap()
    pmt = nc.dram_tensor("pmt", [nb, P, LT], F32, kind="ExternalInput").ap()
    tri8d = nc.dram_tensor("tri8d", [P, P], F8, kind="ExternalInput").ap()
    tri16d = nc.dram_tensor("tri16d", [P, P], F16, kind="ExternalInput").ap()
    out = nc.dram_tensor("out", [nb, L, D], F16, kind="ExternalOutput").ap()

    with tile.TileContext(nc) as tc, ExitStack() as ctx:
        const = ctx.enter_context(tc.tile_pool(name="const", bufs=1))
        xt_pool = ctx.enter_context(tc.tile_pool(name="xt", bufs=3))
        qk_pool = ctx.enter_context(tc.tile_pool(name="qk", bufs=2))
        qk16_pool = ctx.enter_context(tc.tile_pool(name="qk16", bufs=2))
        v_pool = ctx.enter_context(tc.tile_pool(name="v", bufs=2))
        pt_pool = ctx.enter_context(tc.tile_pool(name="pt", bufs=2))
        o_pool = ctx.enter_context(tc.tile_pool(name="o", bufs=6))
        sm_pool = ctx.enter_context(tc.tile_pool(name="sm", bufs=4))
        pm_pool = ctx.enter_context(tc.tile_pool(name="pm", bufs=2))
        proj_ps = ctx.enter_context(tc.tile_pool(name="pps", bufs=3, space="PSUM"))
        s_ps = ctx.enter_context(tc.tile_pool(name="sps", bufs=2, space="PSUM"))
        o_ps = ctx.enter_context(tc.tile_pool(name="ops", bufs=2, space="PSUM"))
        r_ps = ctx.enter_context(tc.tile_pool(name="rps", bufs=1, space="PSUM"))

        # --- constants; on the scalar HWDGE queue so the sync queue is
        # dedicated to x prefetch ---
        wq_sb = const.tile([P, G, 2, D], F8)
        # startup-critical: wq and batch-0 x(lc0) split per-group across all
        # three DMA rings so the first Q group streams in ~4us earlier
        for g in range(3):
            nc.scalar.dma_start(wq_sb[:, g:g + 1],
                                wq8[g:g + 1].rearrange("g p i d -> p g i d"))
        nc.gpsimd.dma_start(wq_sb[:, 3:4],
                            wq8[3:4].rearrange("g p i d -> p g i d"))
        wk_sb = const.tile([P, G, 2, D], F8)
        wv_sb = const.tile([P, G, 2, D], F8)
        wv16_sb = const.tile([P, CT, D], F16)
        wq16_sb = const.tile([P, CT, D], F16)
        wk16_sb = const.tile([P, CT, D], F16)
        bq_sb = const.tile([P, DT], F32)
        nc.scalar.dma_start(bq_sb[:], bq2[:])
        bk_sb = const.tile([P, DT], F32)
        nc.scalar.dma_start(bk_sb[:], bk2[:])
        bv_sb = const.tile([P, D], F32)
        nc.scalar.dma_start(bv_sb[:], bvb[:])
        tri8_sb = const.tile([P, P], F8)
        nc.scalar.dma_start(tri8_sb[:], tri8d[:])
        tri16_sb = const.tile([P, P], F16)
        nc.scalar.dma_start(tri16_sb[:], tri16d[:])

        ones8_sb = const.tile([P, 2, 1], F8)
        nc.vector.memset(ones8_sb[:], 1.0)
        ones16_sb = const.tile([P, 1], F16)
        nc.vector.memset(ones16_sb[:], 1.0)

        # PE warmup: dummy matmuls with no input deps keep the PE busy while
        # batch-0 inputs stream in (HAM clock-gate to 2.4 GHz).
        warm_sb = const.tile([P, 512], F16)
        nc.vector.memset(warm_sb[:], 0.0)
        for w in range(8):
            wps = proj_ps.tile([P, 512], F32, tag="pp", name=f"warm{w}")
            nc.tensor.matmul(wps[:], warm_sb[:, 0:P], warm_sb[:],
                             start=True, stop=True)

        for b in range(nb):
            # --- X^T fp8 pair tiles [128, 2, 1024] ---
            xt = []
            if b == 0:
                # first batch: load the l<512 halves of every tile first so
                # the Q lc=0 groups can start earlier; spread across the
                # sync + gpsimd rings to halve the critical prefetch time
                for g in range(G):
                    t = xt_pool.tile([P, 2, L], F8, tag=f"xt{g}",
                                     name=f"xt{g}_{b}")
                    eng = nc.sync if g < 3 else nc.gpsimd
                    eng.dma_start(t[:, :, 0:512], x8d[b, g, :, :, 0:512])
                    xt.append(t)
                for g in range(G):
                    nc.sync.dma_start(xt[g][:, :, 512:L],
                                      x8d[b, g, :, :, 512:L])
            else:
                for g in range(G):
                    t = xt_pool.tile([P, 2, L], F8, tag=f"xt{g}",
                                     name=f"xt{g}_{b}")
                    nc.sync.dma_start(t[:], x8d[b, g])
                    xt.append(t)
            # fp16 x^T tiles for the l<128 fp16 V island: [128, ct, 128]
            xt16 = xt_pool.tile([P, CT, P], F16, tag="xt16", name=f"xt16_{b}")
            nc.sync.dma_start(xt16[:], x16d[b].rearrange("t p l -> p t l"))
            pm_sb = pm_pool.tile([P, LT], F32, name=f"pm_{b}")
            nc.sync.dma_start(pm_sb[:], pmt[b])
            if b == 0:
                # behind batch-0 x on the sync ring: don't steal HBM bw from
                # the startup-critical x prefetch
                nc.sync.dma_start(wk_sb[:], wk8.rearrange("g p i d -> p g i d"))
                nc.sync.dma_start(wv_sb[:], wv8.rearrange("g p i d -> p g i d"))
                nc.sync.dma_start(wv16_sb[:],
                                  wv16.rearrange("t p d -> p t d"))
                nc.sync.dma_start(wq16_sb[:],
                                  wq16.rearrange("t p d -> p t d"))
                nc.sync.dma_start(wk16_sb[:],
                                  wk16.rearrange("t p d -> p t d"))

            # --- Q^T, K^T: fp8 pair tiles [128, 2, 1024] per d-group;
            # plus fp16 [128d, 128l] tiles of the l<128 slice for S00 ---
            q8t, k8t = [], []
            q16t, k16t = [], []
            for name, w_sb, b_sb, d8 in (("q", wq_sb, bq_sb, q8t),
                                         ("k", wk_sb, bk_sb, k8t)):
                for g2 in range(G2):
                    d8.append(qk_pool.tile([P, 2, L], F8, tag=f"{name}8{g2}",
                                           name=f"{name}8{g2}_{b}"))
                if b == 0 and name == "q":
                    order = [(dt, lc) for lc in range(2) for dt in range(DT)]
                else:
                    order = [(dt, lc) for dt in range(DT) for lc in range(2)]
                for dt, lc in order:
                    ps = proj_ps.tile([P, 512], F32, tag="pp",
                                      name=f"{name}ps{dt}_{lc}_{b}")
                    for g in range(G):
                        nc.tensor.matmul(
                            ps[:],
                            w_sb[:, g, :, dt * P:(dt + 1) * P],
                            xt[g][:, :, lc * 512:(lc + 1) * 512],
                            start=(g == 0), stop=(g == G - 1),
                            perf_mode=DR)
                    # drain on ACT: Identity(ps + bias[d]) -> fp8 pair slice
                    nc.scalar.activation(
                        d8[dt // 2][:, dt % 2, lc * 512:(lc + 1) * 512],
                        ps[:], AF.Identity, bias=b_sb[:, dt:dt + 1])
            # fp16 q,k islands (l<128) from the fp16 path: clean logits for
            # the small-n softmax rows (fp8 proj noise would not cancel)
            for name, w16_sb, b_sb, d16 in (("q", wq16_sb, bq_sb, q16t),
                                            ("k", wk16_sb, bk_sb, k16t)):
                for dt in range(DT):
                    t = qk16_pool.tile([P, P], F16, tag=f"{name}16{dt}",
                                       name=f"{name}16{dt}_{b}")
                    d16.append(t)
                    ps = proj_ps.tile([P, P], F32, tag="pp",
                                      name=f"{name}16ps{dt}_{b}")
                    for ct in range(CT):
                        nc.tensor.matmul(ps[:],
                                         w16_sb[:, ct, dt * P:(dt + 1) * P],
                                         xt16[:, ct, :],
                                         start=(ct == 0), stop=(ct == CT - 1))
                    nc.scalar.activation(t[:], ps[:], AF.Identity,
                                         bias=b_sb[:, dt:dt + 1])

            # --- V: fp8 pair tiles [128, 2, 512] per j-pair; bias on DVE ---
            v8 = []
            for g in range(GL):
                v8.append(v_pool.tile([P, 2, D], F8, tag=f"v{g}",
                                      name=f"v{g}_{b}"))
            for lt in range(LT):
                ps = proj_ps.tile([P, D], F32, tag="pp", name=f"vps{lt}_{b}")
                for g in range(G):
                    nc.tensor.matmul(ps[:], xt[g][:, :, lt * P:(lt + 1) * P],
                                     wv_sb[:, g, :, :],
                                     start=(g == 0), stop=(g == G - 1),
                                     perf_mode=DR)
                nc.vector.tensor_add(v8[lt // 2][:, lt % 2, :], ps[:], bv_sb[:])
            # fp16 V island (rows j<128) from the fp16 path
            v16 = v_pool.tile([P, D], F16, tag="v16", name=f"v16_{b}")
            ps = proj_ps.tile([P, D], F32, tag="pp", name=f"v16ps_{b}")
            for ct in range(CT):
                nc.tensor.matmul(ps[:], xt16[:, ct, :], wv16_sb[:, ct, :],
                                 start=(ct == 0), stop=(ct == CT - 1))
            nc.vector.tensor_add(v16[:], ps[:], bv_sb[:])

            # --- S^T tiles + exp -> P^T fp8 pairs (causal) ---
            p8 = []
            for g in range(GL):
                p8.append(pt_pool.tile([P, 2, L], F8, tag=f"p8{g}",
                                       name=f"p8{g}_{b}"))
            # fp16 S00 island: S^T[j<128, i<128]
            pt16 = pt_pool.tile([P, P], F16, tag="pt16", name=f"pt16_{b}")
            ps = s_ps.tile([P, P], F32, tag="sp", name=f"s00_{b}")
            for dt in range(DT):
                nc.tensor.matmul(ps[:], k16t[dt][:], q16t[dt][:],
                                 start=(dt == 0), stop=(dt == DT - 1))
            nc.scalar.activation(pt16[:], ps[:], AF.Exp,
                                 bias=pm_sb[:, 0:1], scale=SCALE)
            nc.vector.tensor_mul(pt16[:], pt16[:], tri16_sb[:])

            for jb in range(LT):
                j0 = jb * P
                i0 = j0 if jb > 0 else P
                while i0 < L:
                    n = min((i0 // 512 + 1) * 512, L) - i0
                    ps = s_ps.tile([P, n], F32, tag="sp",
                                   name=f"sps{jb}_{i0}_{b}")
                    for g2 in range(G2):
                        nc.tensor.matmul(ps[:],
                                         k8t[g2][:, :, j0:j0 + P],
                                         q8t[g2][:, :, i0:i0 + n],
                                         start=(g2 == 0), stop=(g2 == G2 - 1),
                                         perf_mode=DR)
                    nc.scalar.activation(p8[jb // 2][:, jb % 2, i0:i0 + n],
                                         ps[:], AF.Exp,
                                         bias=pm_sb[:, jb:jb + 1], scale=SCALE)
                    i0 += n
                if jb > 0:
                    # mask the diagonal tile: keep j<=i
                    nc.vector.tensor_mul(p8[jb // 2][:, jb % 2, j0:j0 + P],
                                         p8[jb // 2][:, jb % 2, j0:j0 + P],
                                         tri8_sb[:])
            # zero the causal strips odd planes contribute to even rows:
            # plane jb=2g+1, i-block 2g is never written but IS consumed
            # when output block ib=2g>=2 runs the full pair
            for g in range(1, GL):
                nc.gpsimd.memset(p8[g][:, 1, 2 * g * P:(2 * g + 1) * P], 0.0)

            # --- O' = P^T.T @ V pairs + rowsum piggyback ---
            # all 8 ib rowsums share ONE psum bank, one column each: the
            # accumulation groups hit disjoint [P,1] regions, so ib's
            # rowsum-start never WARs on ib-1's reciprocal (was ~450ns
            # stall, 4x per batch)
            rps = r_ps.tile([P, LT], F32, tag="rp", name=f"rps_{b}")
            for ib in range(LT):
                i0 = ib * P
                rcol = rps[:, ib:ib + 1]
                ops = o_ps.tile([P, D], F32, tag="op", name=f"ops{ib}_{b}")
                # rowsum BEFORE the O matmuls in each pair: its reciprocal
                # then completes during this ib's O work, so ib+1's
                # rowsum-start never stalls on the PE->DVE->PE round-trip
                if ib == 0:
                    nc.tensor.matmul(rcol, pt16[:], ones16_sb[:],
                                     start=True, stop=True)
                    nc.tensor.matmul(ops[:], pt16[:], v16[:],
                                     start=True, stop=True)
                else:
                    ng = (ib + 2) // 2  # pairs covering j-tiles 0..ib (+pad)
                    for g in range(ng):
                        pT = p8[g][:, :, i0:i0 + P]
                        nc.tensor.matmul(rcol, pT, ones8_sb[:],
                                         start=(g == 0), stop=(g == ng - 1),
                                         perf_mode=DR)
                        nc.tensor.matmul(ops[:], pT, v8[g][:],
                                         start=(g == 0), stop=(g == ng - 1),
                                         perf_mode=DR)
                rec = sm_pool.tile([P, 1], F32, tag="rec", name=f"rec{ib}_{b}")
                nc.vector.reciprocal(rec[:], rcol)
                o_sb = o_pool.tile([P, D], F16, tag="ot", name=f"o{ib}_{b}")
                # relu(O'/rowsum): alternate ACT/DVE so the 8-deep normalize
                # burst at the PV tail doesn't queue on one engine and gate
                # the o-psum rotation
                if ib % 2 == 0:
                    nc.scalar.activation(o_sb[:], ops[:], AF.Relu,
                                         scale=rec[:, 0:1])
                else:
                    nc.vector.tensor_scalar(o_sb[:], ops[:], rec[:], 0.0,
                                            mybir.AluOpType.mult,
                                            mybir.AluOpType.max)
                # SWDGE so stores never head-of-line-block the x prefetch;
                # last batch has no prefetch left, so use the two idle HWDGE
                # rings alternately to halve the final store drain
                if b == nb - 1:
                    eng = nc.sync if ib % 2 == 0 else nc.scalar
                    eng.dma_start(out[b, i0:i0 + P, :], o_sb[:])
                else:
                    nc.gpsimd.dma_start(out[b, i0:i0 + P, :], o_sb[:])

    nc.compile()
    return nc


def _prep_host(x, Wq, bq, Wk, bk, Wv, bv, mask):
    # x^T pairs: x8d[b, g, p, i, l] = x[b, l, 256g+128i+p] in e4m3
    x8 = x.astype(E4NP)
    x8d = np.ascontiguousarray(
        x8.transpose(0, 2, 1).reshape(B, G, 2, P, L).transpose(0, 1, 3, 2, 4))
    x16d = np.ascontiguousarray(
        x[:, 0:P, :].astype(np.float16).transpose(0, 2, 1).reshape(
            B, CT, P, P))
    # weights: w8[g, p, i, d] = W[d, 256g+128i+p]
    def w8of(W):
        return np.ascontiguousarray(
            W.T.astype(E4NP).reshape(G, 2, P, D).transpose(0, 2, 1, 3))
    wq8 = w8of(Wq)
    wk8 = w8of(Wk)
    wv8 = w8of(Wv)
    wv16 = np.ascontiguousarray(
        Wv.T.astype(np.float16).reshape(CT, P, D))
    wq16 = np.ascontiguousarray(
        Wq.T.astype(np.float16).reshape(CT, P, D))
    wk16 = np.ascontiguousarray(
        Wk.T.astype(np.float16).reshape(CT, P, D))
    bq2 = np.ascontiguousarray(bq.astype(np.float32).reshape(DT, P).T)
    bk2 = np.ascontiguousarray(bk.astype(np.float32).reshape(DT, P).T)
    bvb = np.ascontiguousarray(
        np.broadcast_to(bv.astype(np.float32), (P, D)))
    pm = np.where(mask[:, 0, :] != 0, 0.0, NEG).astype(np.float32)  # [B, L]
    pmt = np.ascontiguousarray(pm.reshape(B, LT, P).transpose(0, 2, 1))
    tri = (np.arange(P)[:, None] <= np.arange(P)[None, :])
    tri8 = tri.astype(E4NP)
    tri16 = tri.astype(np.float16)
    return (x8d, x16d, wq8, wk8, wv8, wv16, wq16, wk16, bq2, bk2, bvb, pmt,
            tri8, tri16)


_NC_CACHE = {}


def kernel(x, Wq, bq, Wk, bk, Wv, bv, mask):
    x = np.asarray(x)
    Wq, bq = np.asarray(Wq), np.asarray(bq)
    Wk, bk = np.asarray(Wk), np.asarray(bk)
    Wv, bv = np.asarray(Wv), np.asarray(bv)
    mask = np.asarray(mask)

    (x8d, x16d, wq8, wk8, wv8, wv16, wq16, wk16, bq2, bk2, bvb, pmt, tri8,
     tri16) = _prep_host(x, Wq, bq, Wk, bk, Wv, bv, mask)

    if "nc" not in _NC_CACHE:
        _NC_CACHE["nc"] = build_program(NB)
    nc = _NC_CACHE["nc"]

    in_maps = []
    for c in range(N_CORES):
        s = slice(c * NB, (c + 1) * NB)
        in_maps.append({
            "x8d": np.ascontiguousarray(x8d[s]),
            "x16d": np.ascontiguousarray(x16d[s]),
            "wq8": wq8, "wk8": wk8, "wv8": wv8, "wv16": wv16,
            "wq16": wq16, "wk16": wk16,
            "bq2": bq2, "bk2": bk2, "bvb": bvb,
            "pmt": np.ascontiguousarray(pmt[s]),
            "tri8d": tri8, "tri16d": tri16,
        })

    res = bass_utils.run_bass_kernel_spmd(
        nc, in_maps, core_ids=list(range(N_CORES)),
        trace=bool(int(os.environ.get("KERNEL_TRACE", "0"))),
    )
    if os.environ.get("KERNEL_RESULT_HOOK"):
        _NC_CACHE["last_result"] = res

    return np.concatenate(
        [res.results[c]["out"] for c in range(N_CORES)],
        axis=0).astype(np.float32)


# revision 51
# speedup vs baseline: 1.0232x; 1.0113x over previous
"""Causal attention (QKV proj + softmax + PV + ReLU) on 8 trn2 NeuronCores.

Sharding: data-parallel over batch B=32 -> 4 batches per core; projection
weights replicated.

Numerics: fp8-e4m3 with perf_mode=DoubleRow (contraction 256 per pass,
~1.8x the fp16 PE issue rate; N=512 DR matmuls measured at ~215-231 ns,
LDWEIGHTS fully hidden by the PE reorder window) for the bulk of the FLOPs;
fp32 PSUM accumulation. Max-norm accuracy is protected by small fp16
islands for the early rows, where softmax averages over few values and fp8
noise would not cancel (numpy-simulated: full-fp8 rel err 6.1e-2 FAILs;
fp8 with islands 7.5e-3; measured on HW 7.57e-3 vs the 2e-2 gate):
  - rows i<128 get fp16 S from fp16-PROJECTED q,k islands (the dominant
    term is x8/W8 projection noise, so re-draining the fp8 psum at fp16
    is NOT enough), fp16 P00, and a fp16 V tile
  - everything else runs q8/k8/p8/v8 through DoubleRow pairs
The softmax rowsum is computed from the SAME quantized P used for P@V, so
uniform quantization error cancels in the normalize.

Per core, per batch:
  Q^T,K^T[d,l] = W8^T.T @ X8^T   (4 DoubleRow matmuls, 256-contraction each;
      bias folded into the ACT Identity drain: per-partition bias = b[d])
  V[l,d] = X8^T.T @ Wv8^T        (DoubleRow; bias via DVE add during drain)
  S^T[j,i] tiles = K8^T.T @ Q8^T  (2 DoubleRow matmuls per tile; causal:
      chunks start at the diagonal; (j<128,i<128) tile comes from fp16)
  P^T = exp(scale*S^T + padmask_bias_j) via one ACT op -> fp8 pair tiles;
      diagonal tile masked by a DVE multiply; the causal-zero 128-col strip
      of odd planes is memset so even output rows can consume full pairs
  O' = P8^T.T @ V8 pairs, rowsum = P8^T.T @ ones8 piggybacks on the same
      stationary operand (LDWEIGHTS deduped)
  out = Relu(O' * (1/rowsum)) on ACT (per-partition scale), fp16, DMA out

Engine balance (NTFF-verified): Q/K proj drains + exps + half the PV
normalizes on ACT (per-partition bias/scale), V drains + diag masks +
reciprocals + the other normalizes on DVE, stores + P-zero memsets on
gpsimd, x prefetch on sync, weights on scalar -- no engine above ~65% of
the PE's fp8 critical path. PSUM: proj pool 3 banks, S 2, PV-O 2,
rowsum 1 (rowsum drains fast via reciprocal, so 1 bank suffices; the
freed bank decouples proj drains from the matmul stream). o_sb bufs=6 so
the PV normalize burst never waits on store DMAs. Warmup matmuls pre-warm
the HAM clock-gate while wq8 (split per-group over the scalar+gpsimd
rings) and batch-0 x lc0-halves (sync+gpsimd) stream in.
"""

import os
from contextlib import ExitStack

import numpy as np
import ml_dtypes

import concourse.tile as tile
from concourse import bacc, mybir
from concourse import bass_utils

F32 = mybir.dt.float32
F16 = mybir.dt.float16
F8 = mybir.dt.float8e4
AF = mybir.ActivationFunctionType
DR = mybir.MatmulPerfMode.DoubleRow
E4NP = ml_dtypes.float8_e4m3

N_CORES = 8
B = 32
L = 1024
C = 1024  # d_model
D = 512
P = 128
NB = B // N_CORES  # batches per core
CT = C // P  # 8 c-tiles
G = C // 256  # 4 DoubleRow contraction groups over c
DT = D // P  # 4 d tiles
G2 = D // 256  # 2 DoubleRow groups over d
LT = L // P  # 8 l/j/i tiles
GL = LT // 2  # 4 j-pair groups
SCALE = float(D) ** -0.5
NEG = -30000.0


def build_program(nb: int = NB):
    """Build the per-core Bass program for nb batches."""
    nc = bacc.Bacc("TRN2", target_bir_lowering=False, debug=False,
                   num_devices=N_CORES)

    # [b, g, p, i, l] fp8 pairs of x^T; plane i pairs c = 256g+128i+p
    x8d = nc.dram_tensor("x8d", [nb, G, P, 2, L], F8, kind="ExternalInput").ap()
    # [b, ct, p, l<128] fp16 x^T for the l<128 fp16 V island
    x16d = nc.dram_tensor("x16d", [nb, CT, P, P], F16, kind="ExternalInput").ap()
    wq8 = nc.dram_tensor("wq8", [G, P, 2, D], F8, kind="ExternalInput").ap()
    wk8 = nc.dram_tensor("wk8", [G, P, 2, D], F8, kind="ExternalInput").ap()
    wv8 = nc.dram_tensor("wv8", [G, P, 2, D], F8, kind="ExternalInput").ap()
    wv16 = nc.dram_tensor("wv16", [CT, P, D], F16, kind="ExternalInput").ap()
    wq16 = nc.dram_tensor("wq16", [CT, P, D], F16, kind="ExternalInput").ap()
    wk16 = nc.dram_tensor("wk16", [CT, P, D], F16, kind="ExternalInput").ap()
    bq2 = nc.dram_tensor("bq2", [P, DT], F32, kind="ExternalInput").ap()
    bk2 = nc.dram_tensor("bk2", [P, DT], F32, kind="ExternalInput").ap()
    bvb = nc.dram_tensor("bvb", [P, D], F32, kind="ExternalInput").ap()
    pmt = nc.dram_tensor("pmt", [nb, P, LT], F32, kind="ExternalInput").ap()
    tri8d = nc.dram_tensor("tri8d", [P, P], F8, kind="ExternalInput").ap()
    tri16d = nc.dram_tensor("tri16d", [P, P], F16, kind="ExternalInput").ap()
    out = nc.dram_tensor("out", [nb, L, D], F16, kind="ExternalOutput").ap()

    with tile.TileContext(nc) as tc, ExitStack() as ctx:
        const = ctx.enter_context(tc.tile_pool(name="const", bufs=1))
        xt_pool = ctx.enter_context(tc.tile_pool(name="xt", bufs=3))
        qk_pool = ctx.enter_context(tc.tile_pool(name="qk", bufs=2))
        qk16_pool = ctx.enter_context(tc.tile_pool(name="qk16", bufs=2))
        v_pool = ctx.enter_context(tc.tile_pool(name="v", bufs=2))
        pt_pool = ctx.enter_context(tc.tile_pool(name="pt", bufs=2))
        o_pool = ctx.enter_context(tc.tile_pool(name="o", bufs=6))
        sm_pool = ctx.enter_context(tc.tile_pool(name="sm", bufs=4))
        pm_pool = ctx.enter_context(tc.tile_pool(name="pm", bufs=2))
        proj_ps = ctx.enter_context(tc.tile_pool(name="pps", bufs=3, space="PSUM"))
        s_ps = ctx.enter_context(tc.tile_pool(name="sps", bufs=2, space="PSUM"))
        o_ps = ctx.enter_context(tc.tile_pool(name="ops", bufs=2, space="PSUM"))
        r_ps = ctx.enter_context(tc.tile_pool(name="rps", bufs=1, space="PSUM"))

        # --- constants; on the scalar HWDGE queue so the sync queue is
        # dedicated to x prefetch ---
        wq_sb = const.tile([P, G, 2, D], F8)
        # startup-critical: wq and batch-0 x(lc0) split per-group across all
        # three DMA rings so the first Q group streams in ~4us earlier
        for g in range(3):
            nc.scalar.dma_start(wq_sb[:, g:g + 1],
                                wq8[g:g + 1].rearrange("g p i d -> p g i d"))
        nc.gpsimd.dma_start(wq_sb[:, 3:4],
                            wq8[3:4].rearrange("g p i d -> p g i d"))
        wk_sb = const.tile([P, G, 2, D], F8)
        wv_sb = const.tile([P, G, 2, D], F8)
        wv16_sb = const.tile([P, CT, D], F16)
        wq16_sb = const.tile([P, CT, D], F16)
        wk16_sb = const.tile([P, CT, D], F16)
        bq_sb = const.tile([P, DT], F32)
        nc.scalar.dma_start(bq_sb[:], bq2[:])
        bk_sb = const.tile([P, DT], F32)
        nc.scalar.dma_start(bk_sb[:], bk2[:])
        bv_sb = const.tile([P, D], F32)
        nc.scalar.dma_start(bv_sb[:], bvb[:])
        tri8_sb = const.tile([P, P], F8)
        nc.scalar.dma_start(tri8_sb[:], tri8d[:])
        tri16_sb = const.tile([P, P], F16)
        nc.scalar.dma_start(tri16_sb[:], tri16d[:])

        ones8_sb = const.tile([P, 2, 1], F8)
        nc.vector.memset(ones8_sb[:], 1.0)
        ones16_sb = const.tile([P, 1], F16)
        nc.vector.memset(ones16_sb[:], 1.0)

        # PE warmup: dummy matmuls with no input deps keep the PE busy while
        # batch-0 inputs stream in (HAM clock-gate to 2.4 GHz).
        warm_sb = const.tile([P, 512], F16)
        nc.vector.memset(warm_sb[:], 0.0)
        for w in range(8):
            wps = proj_ps.tile([P, 512], F32, tag="pp", name=f"warm{w}")
            nc.tensor.matmul(wps[:], warm_sb[:, 0:P], warm_sb[:],
                             start=True, stop=True)

        for b in range(nb):
            # --- X^T fp8 pair tiles [128, 2, 1024] ---
            xt = []
            if b == 0:
                # first batch: load the l<512 halves of every tile first so
                # the Q lc=0 groups can start earlier; spread across the
                # sync + gpsimd rings to halve the critical prefetch time
                for g in range(G):
                    t = xt_pool.tile([P, 2, L], F8, tag=f"xt{g}",
                                     name=f"xt{g}_{b}")
                    eng = nc.sync if g < 3 else nc.gpsimd
                    eng.dma_start(t[:, :, 0:512], x8d[b, g, :, :, 0:512])
                    xt.append(t)
                for g in range(G):
                    nc.sync.dma_start(xt[g][:, :, 512:L],
                                      x8d[b, g, :, :, 512:L])
            else:
                for g in range(G):
                    t = xt_pool.tile([P, 2, L], F8, tag=f"xt{g}",
                                     name=f"xt{g}_{b}")
                    nc.sync.dma_start(t[:], x8d[b, g])
                    xt.append(t)
            # fp16 x^T tiles for the l<128 fp16 V island: [128, ct, 128]
            xt16 = xt_pool.tile([P, CT, P], F16, tag="xt16", name=f"xt16_{b}")
            nc.sync.dma_start(xt16[:], x16d[b].rearrange("t p l -> p t l"))
            pm_sb = pm_pool.tile([P, LT], F32, name=f"pm_{b}")
            nc.sync.dma_start(pm_sb[:], pmt[b])
            if b == 0:
                # behind batch-0 x on the sync ring: don't steal HBM bw from
                # the startup-critical x prefetch
                nc.sync.dma_start(wk_sb[:], wk8.rearrange("g p i d -> p g i d"))
                nc.sync.dma_start(wv_sb[:], wv8.rearrange("g p i d -> p g i d"))
                nc.sync.dma_start(wv16_sb[:],
                                  wv16.rearrange("t p d -> p t d"))
                nc.sync.dma_start(wq16_sb[:],
                                  wq16.rearrange("t p d -> p t d"))
                nc.sync.dma_start(wk16_sb[:],
                                  wk16.rearrange("t p d -> p t d"))

            # --- Q^T, K^T: fp8 pair tiles [128, 2, 1024] per d-group;
            # plus fp16 [128d, 128l] tiles of the l<128 slice for S00 ---
            q8t, k8t = [], []
            q16t, k16t = [], []
            for name, w_sb, b_sb, d8 in (("q", wq_sb, bq_sb, q8t),
                                         ("k", wk_sb, bk_sb, k8t)):
                for g2 in range(G2):
                    d8.append(qk_pool.tile([P, 2, L], F8, tag=f"{name}8{g2}",
                                           name=f"{name}8{g2}_{b}"))
                if b == 0 and name == "q":
                    order = [(dt, lc) for lc in range(2) for dt in range(DT)]
                else:
                    order = [(dt, lc) for dt in range(DT) for lc in range(2)]
                for dt, lc in order:
                    ps = proj_ps.tile([P, 512], F32, tag="pp",
                                      name=f"{name}ps{dt}_{lc}_{b}")
                    for g in range(G):
                        nc.tensor.matmul(
                            ps[:],
                            w_sb[:, g, :, dt * P:(dt + 1) * P],
                            xt[g][:, :, lc * 512:(lc + 1) * 512],
                            start=(g == 0), stop=(g == G - 1),
                            perf_mode=DR)
                    # drain on ACT: Identity(ps + bias[d]) -> fp8 pair slice.
                    # Batch 0's K drains go to the then-idle DVE instead, so
                    # the pipeline-fill isn't serialized on the ACT queue
                    # (steady-state DVE drains measured slower -- batch 0 only)
                    if b == 0 and name == "k":
                        nc.vector.tensor_scalar_add(
                            d8[dt // 2][:, dt % 2, lc * 512:(lc + 1) * 512],
                            ps[:], b_sb[:, dt:dt + 1])
                    else:
                        nc.scalar.activation(
                            d8[dt // 2][:, dt % 2, lc * 512:(lc + 1) * 512],
                            ps[:], AF.Identity, bias=b_sb[:, dt:dt + 1])
            # fp16 q,k islands (l<128) from the fp16 path: clean logits for
            # the small-n softmax rows (fp8 proj noise would not cancel)
            for name, w16_sb, b_sb, d16 in (("q", wq16_sb, bq_sb, q16t),
                                            ("k", wk16_sb, bk_sb, k16t)):
                for dt in range(DT):
                    t = qk16_pool.tile([P, P], F16, tag=f"{name}16{dt}",
                                       name=f"{name}16{dt}_{b}")
                    d16.append(t)
                    ps = proj_ps.tile([P, P], F32, tag="pp",
                                      name=f"{name}16ps{dt}_{b}")
                    for ct in range(CT):
                        nc.tensor.matmul(ps[:],
                                         w16_sb[:, ct, dt * P:(dt + 1) * P],
                                         xt16[:, ct, :],
                                         start=(ct == 0), stop=(ct == CT - 1))
                    nc.scalar.activation(t[:], ps[:], AF.Identity,
                                         bias=b_sb[:, dt:dt + 1])

            # --- V: fp8 pair tiles [128, 2, 512] per j-pair; bias on DVE ---
            v8 = []
            for g in range(GL):
                v8.append(v_pool.tile([P, 2, D], F8, tag=f"v{g}",
                                      name=f"v{g}_{b}"))
            for lt in range(LT):
                ps = proj_ps.tile([P, D], F32, tag="pp", name=f"vps{lt}_{b}")
                for g in range(G):
                    nc.tensor.matmul(ps[:], xt[g][:, :, lt * P:(lt + 1) * P],
                                     wv_sb[:, g, :, :],
                                     start=(g == 0), stop=(g == G - 1),
                                     perf_mode=DR)
                nc.vector.tensor_add(v8[lt // 2][:, lt % 2, :], ps[:], bv_sb[:])
            # fp16 V island (rows j<128) from the fp16 path
            v16 = v_pool.tile([P, D], F16, tag="v16", name=f"v16_{b}")
            ps = proj_ps.tile([P, D], F32, tag="pp", name=f"v16ps_{b}")
            for ct in range(CT):
                nc.tensor.matmul(ps[:], xt16[:, ct, :], wv16_sb[:, ct, :],
                                 start=(ct == 0), stop=(ct == CT - 1))
            nc.vector.tensor_add(v16[:], ps[:], bv_sb[:])

            # --- S^T tiles + exp -> P^T fp8 pairs (causal) ---
            p8 = []
            for g in range(GL):
                p8.append(pt_pool.tile([P, 2, L], F8, tag=f"p8{g}",
                                       name=f"p8{g}_{b}"))
            # fp16 S00 island: S^T[j<128, i<128]
            pt16 = pt_pool.tile([P, P], F16, tag="pt16", name=f"pt16_{b}")
            ps = s_ps.tile([P, P], F32, tag="sp", name=f"s00_{b}")
            for dt in range(DT):
                nc.tensor.matmul(ps[:], k16t[dt][:], q16t[dt][:],
                                 start=(dt == 0), stop=(dt == DT - 1))
            nc.scalar.activation(pt16[:], ps[:], AF.Exp,
                                 bias=pm_sb[:, 0:1], scale=SCALE)
            nc.vector.tensor_mul(pt16[:], pt16[:], tri16_sb[:])

            for jb in range(LT):
                j0 = jb * P
                i0 = j0 if jb > 0 else P
                while i0 < L:
                    n = min((i0 // 512 + 1) * 512, L) - i0
                    ps = s_ps.tile([P, n], F32, tag="sp",
                                   name=f"sps{jb}_{i0}_{b}")
                    for g2 in range(G2):
                        nc.tensor.matmul(ps[:],
                                         k8t[g2][:, :, j0:j0 + P],
                                         q8t[g2][:, :, i0:i0 + n],
                                         start=(g2 == 0), stop=(g2 == G2 - 1),
                                         perf_mode=DR)
                    nc.scalar.activation(p8[jb // 2][:, jb % 2, i0:i0 + n],
                                         ps[:], AF.Exp,
                                         bias=pm_sb[:, jb:jb + 1], scale=SCALE)
                    i0 += n
                if jb > 0:
                    # mask the diagonal tile: keep j<=i
                    nc.vector.tensor_mul(p8[jb // 2][:, jb % 2, j0:j0 + P],
                                         p8[jb // 2][:, jb % 2, j0:j0 + P],
                                         tri8_sb[:])
            # zero the causal strips odd planes contribute to even rows:
            # plane jb=2g+1, i-block 2g is never written but IS consumed
            # when output block ib=2g>=2 runs the full pair
            for g in range(1, GL):
                nc.gpsimd.memset(p8[g][:, 1, 2 * g * P:(2 * g + 1) * P], 0.0)

            # --- O' = P^T.T @ V pairs + rowsum piggyback ---
            # all 8 ib rowsums share ONE psum bank, one column each: the
            # accumulation groups hit disjoint [P,1] regions, so ib's
            # rowsum-start never WARs on ib-1's reciprocal (was ~450ns
            # stall, 4x per batch)
            rps = r_ps.tile([P, LT], F32, tag="rp", name=f"rps_{b}")
            for ib in range(LT):
                i0 = ib * P
                rcol = rps[:, ib:ib + 1]
                ops = o_ps.tile([P, D], F32, tag="op", name=f"ops{ib}_{b}")
                # rowsum BEFORE the O matmuls in each pair: its reciprocal
                # then completes during this ib's O work, so ib+1's
                # rowsum-start never stalls on the PE->DVE->PE round-trip
                if ib == 0:
                    nc.tensor.matmul(rcol, pt16[:], ones16_sb[:],
                                     start=True, stop=True)
                    nc.tensor.matmul(ops[:], pt16[:], v16[:],
                                     start=True, stop=True)
                else:
                    ng = (ib + 2) // 2  # pairs covering j-tiles 0..ib (+pad)
                    for g in range(ng):
                        pT = p8[g][:, :, i0:i0 + P]
                        nc.tensor.matmul(rcol, pT, ones8_sb[:],
                                         start=(g == 0), stop=(g == ng - 1),
                                         perf_mode=DR)
                        nc.tensor.matmul(ops[:], pT, v8[g][:],
                                         start=(g == 0), stop=(g == ng - 1),
                                         perf_mode=DR)
                rec = sm_pool.tile([P, 1], F32, tag="rec", name=f"rec{ib}_{b}")
                nc.vector.reciprocal(rec[:], rcol)
                o_sb = o_pool.tile([P, D], F16, tag="ot", name=f"o{ib}_{b}")
                # relu(O'/rowsum): alternate ACT/DVE so the 8-deep normalize
                # burst at the PV tail doesn't queue on one engine and gate
                # the o-psum rotation
                if ib % 2 == 0:
                    nc.scalar.activation(o_sb[:], ops[:], AF.Relu,
                                         scale=rec[:, 0:1])
                else:
                    nc.vector.tensor_scalar(o_sb[:], ops[:], rec[:], 0.0,
                                            mybir.AluOpType.mult,
                                            mybir.AluOpType.max)
                # SWDGE so stores never head-of-line-block the x prefetch;
                # last batch has no prefetch left, so use the two idle HWDGE
                # rings alternately to halve the final store drain
                if b == nb - 1:
                    eng = nc.sync if ib % 2 == 0 else nc.scalar
                    eng.dma_start(out[b, i0:i0 + P, :], o_sb[:])
                else:
                    nc.gpsimd.dma_start(out[b, i0:i0 + P, :], o_sb[:])

    nc.compile()
    return nc


def _prep_host(x, Wq, bq, Wk, bk, Wv, bv, mask):
    # x^T pairs: x8d[b, g, p, i, l] = x[b, l, 256g+128i+p] in e4m3
    x8 = x.astype(E4NP)
    x8d = np.ascontiguousarray(
        x8.transpose(0, 2, 1).reshape(B, G, 2, P, L).transpose(0, 1, 3, 2, 4))
    x16d = np.ascontiguousarray(
        x[:, 0:P, :].astype(np.float16).transpose(0, 2, 1).reshape(
            B, CT, P, P))
    # weights: w8[g, p, i, d] = W[d, 256g+128i+p]
    def w8of(W):
        return np.ascontiguousarray(
            W.T.astype(E4NP).reshape(G, 2, P, D).transpose(0, 2, 1, 3))
    wq8 = w8of(Wq)
    wk8 = w8of(Wk)
    wv8 = w8of(Wv)
    wv16 = np.ascontiguousarray(
        Wv.T.astype(np.float16).reshape(CT, P, D))
    wq16 = np.ascontiguousarray(
        Wq.T.astype(np.float16).reshape(CT, P, D))
    wk16 = np.ascontiguousarray(
        Wk.T.astype(np.float16).reshape(CT, P, D))
    bq2 = np.ascontiguousarray(bq.astype(np.float32).reshape(DT, P).T)
    bk2 = np.ascontiguousarray(bk.astype(np.float32).reshape(DT, P).T)
    bvb = np.ascontiguousarray(
        np.broadcast_to(bv.astype(np.float32), (P, D)))
    pm = np.where(mask[:, 0, :] != 0, 0.0, NEG).astype(np.float32)  # [B, L]
    pmt = np.ascontiguousarray(pm.reshape(B, LT, P).transpose(0, 2, 1))
    tri = (np.arange(P)[:, None] <= np.arange(P)[None, :])
    tri8 = tri.astype(E4NP)
    tri16 = tri.astype(np.float16)
    return (x8d, x16d, wq8, wk8, wv8, wv16, wq16, wk16, bq2, bk2, bvb, pmt,
            tri8, tri16)


_NC_CACHE = {}


def kernel(x, Wq, bq, Wk, bk, Wv, bv, mask):
    x = np.asarray(x)
    Wq, bq = np.asarray(Wq), np.asarray(bq)
    Wk, bk = np.asarray(Wk), np.asarray(bk)
    Wv, bv = np.asarray(Wv), np.asarray(bv)
    mask = np.asarray(mask)

    (x8d, x16d, wq8, wk8, wv8, wv16, wq16, wk16, bq2, bk2, bvb, pmt, tri8,
     tri16) = _prep_host(x, Wq, bq, Wk, bk, Wv, bv, mask)

    if "nc" not in _NC_CACHE:
        _NC_CACHE["nc"] = build_program(NB)
    nc = _NC_CACHE["nc"]

    in_maps = []
    for c in range(N_CORES):
        s = slice(c * NB, (c + 1) * NB)
        in_maps.append({
            "x8d": np.ascontiguousarray(x8d[s]),
            "x16d": np.ascontiguousarray(x16d[s]),
            "wq8": wq8, "wk8": wk8, "wv8": wv8, "wv16": wv16,
            "wq16": wq16, "wk16": wk16,
            "bq2": bq2, "bk2": bk2, "bvb": bvb,
            "pmt": np.ascontiguousarray(pmt[s]),
            "tri8d": tri8, "tri16d": tri16,
        })

    res = bass_utils.run_bass_kernel_spmd(
        nc, in_maps, core_ids=list(range(N_CORES)),
        trace=bool(int(os.environ.get("KERNEL_TRACE", "0"))),
    )
    if os.environ.get("KERNEL_RESULT_HOOK"):
        _NC_CACHE["last_result"] = res

    return np.concatenate(
        [res.results[c]["out"] for c in range(N_CORES)],
        axis=0).astype(np.float32)
